# revision 21
# baseline (speedup 1.0000x reference)
"""Trainium2 Bass kernel for BoxHead (nn_BoxHead_33277406609979).

Computes, for feature_vectors [8000, 12544] (fp32):
    h  = relu(x @ W1 + b1)          # [N, 1024]
    h  = relu(h @ W2 + b2)          # [N, 1024]
    cp = softmax(h @ Wc + bc)       # [N, 4]
    bp = h @ Wr + br                # [N, 12]

Strategy: data-parallel over the proposal dim N. 8000 rows are padded to
8192 and split 1024/core across 8 NeuronCores; the MLP weights are
replicated. On-device everything is computed in "transposed activation"
layout (h^T tiles [ch, box]) so the natural row-major layouts of W1/W2
serve directly as the stationary matmul operand and no on-device
transposes are needed. The host pre-transposes each X shard once
(contraction dim must lie on SBUF partitions). Matmuls run as float32r
(fp32 rounded to e8m11; full-rate 1 cycle/row on the PE vs 4 for fp32)
accumulating in fp32 PSUM.

Built on bacc.Bacc (not raw bass.Bass): Bacc.compile() runs
generate_event_semaphores, which splits multi-sem waits to satisfy the
TRN2 1-wait-per-instruction ISA constraint.
"""

import numpy as np

import concourse.bass as bass
import concourse.bacc as bacc
import concourse.mybir as mybir
import concourse.tile as tile
from concourse.bass_utils import run_bass_kernel_spmd

N_CORES = 8
D_IN = 12544   # 98 k-tiles of 128
H = 1024       # 8 tiles of 128
N_BOX = 8000
B = 1024       # padded boxes per core
C_OUT = 16     # 4 class logits + 12 regression outputs

F32 = mybir.dt.float32
F32R = mybir.dt.float32r
RELU = mybir.ActivationFunctionType.Relu
EXP = mybir.ActivationFunctionType.Exp
AX = mybir.AxisListType.X
MAX = mybir.AluOpType.max
ADD = mybir.AluOpType.add


def build_program(d_in: int = D_IN, h_dim: int = H, b: int = B) -> bass.Bass:
    """Build the per-core Bass program (SPMD: same program on all cores)."""
    kd = d_in // 128   # contraction tiles for layer 1
    kh = h_dim // 128  # tiles along the hidden dim
    bh = min(512, b)   # box-half size (one PSUM bank of fp32)
    n_half = b // bh

    nc = bacc.Bacc(trn_type="TRN2", target_bir_lowering=False, debug=False)

    xt = nc.dram_tensor("xt", [d_in, b], F32R, kind="ExternalInput").ap()
    w1 = nc.dram_tensor("w1", [d_in, h_dim], F32R, kind="ExternalInput").ap()
    w2 = nc.dram_tensor("w2", [h_dim, h_dim], F32R, kind="ExternalInput").ap()
    wcr = nc.dram_tensor("wcr", [h_dim, C_OUT], F32, kind="ExternalInput").ap()
    b1d = nc.dram_tensor("b1", [h_dim], F32, kind="ExternalInput").ap()
    b2d = nc.dram_tensor("b2", [h_dim], F32, kind="ExternalInput").ap()
    bcrd = nc.dram_tensor("bcr", [128, C_OUT], F32, kind="ExternalInput").ap()
    out = nc.dram_tensor("out", [b, C_OUT], F32, kind="ExternalOutput").ap()

    with tile.TileContext(nc) as tc:
        _body(tc, xt, w1, w2, wcr, b1d, b2d, bcrd, out, kd, kh, bh, n_half)
    nc.compile()
    return nc


def _body(tc, xt, w1, w2, wcr, b1d, b2d, bcrd, out, kd, kh, bh, n_half):
    nc = tc.nc
    h_dim = kh * 128
    with (
        tc.tile_pool(name="consts", bufs=1) as consts,
        tc.tile_pool(name="stream", bufs=4) as stream,
        tc.tile_pool(name="acts", bufs=1) as acts,
        tc.tile_pool(name="psum", bufs=8, space="PSUM") as psum_pool,
        tc.tile_pool(name="outp", bufs=4) as outp,
    ):
        # --- persistent weights / biases in SBUF ---
        # w2_sb columns: k*h_dim + ch2  (partition = ch1 within k-tile)
        w2_sb = consts.tile([128, kh * h_dim], F32R)
        for k in range(kh):
            nc.sync.dma_start(
                w2_sb[:, k * h_dim : (k + 1) * h_dim],
                w2[k * 128 : (k + 1) * 128, :],
            )
        # wcr_sb columns: k*C_OUT + c (partition = ch2 within k-tile)
        wcr_sb = consts.tile([128, kh * C_OUT], F32)
        for k in range(kh):
            nc.sync.dma_start(
                wcr_sb[:, k * C_OUT : (k + 1) * C_OUT],
                wcr[k * 128 : (k + 1) * 128, :],
            )
        # b1_sb[p, c] = b1[c*128 + p]
        b1_sb = consts.tile([128, kh], F32)
        nc.sync.dma_start(b1_sb[:], b1d.rearrange("(c p) -> p c", p=128))
        b2_sb = consts.tile([128, kh], F32)
        nc.sync.dma_start(b2_sb[:], b2d.rearrange("(c p) -> p c", p=128))
        bcr_sb = consts.tile([128, C_OUT], F32)
        nc.sync.dma_start(bcr_sb[:], bcrd[:])

        # transposed activations, columns: k*B + box
        # h1 is written as fp32r by the ReLU so layer 2 can matmul it at full rate
        h1 = acts.tile([128, kh * (bh * n_half)], F32R)
        h2 = acts.tile([128, kh * (bh * n_half)], F32)
        b_total = bh * n_half

        for half in range(n_half):
            bs = half * bh
            # ---- layer 1: h1[ch, box-half] = relu(W1^T @ X^T + b1) ----
            ps1 = [
                psum_pool.tile([128, bh], F32, tag="acc", name=f"ps1_{half}_{c}")
                for c in range(kh)
            ]
            for d in range(kd):
                xt_t = stream.tile([128, bh], F32R, tag="xt", name="xt_t")
                nc.sync.dma_start(xt_t[:], xt[d * 128 : (d + 1) * 128, bs : bs + bh])
                w1_t = stream.tile([128, h_dim], F32R, tag="w1", name="w1_t")
                nc.sync.dma_start(w1_t[:], w1[d * 128 : (d + 1) * 128, :])
                for c in range(kh):
                    nc.tensor.matmul(
                        ps1[c][:],
                        w1_t[:, c * 128 : (c + 1) * 128],
                        xt_t[:],
                        start=(d == 0),
                        stop=(d == kd - 1),
                    )
            for c in range(kh):
                nc.scalar.activation(
                    h1[:, c * b_total + bs : c * b_total + bs + bh],
                    ps1[c][:],
                    RELU,
                    bias=b1_sb[:, c : c + 1],
                )

            # ---- layer 2: h2 = relu(W2^T @ h1 + b2) ----
            ps2 = [
                psum_pool.tile([128, bh], F32, tag="acc", name=f"ps2_{half}_{c}")
                for c in range(kh)
            ]
            for c2 in range(kh):
                for k in range(kh):
                    nc.tensor.matmul(
                        ps2[c2][:],
                        w2_sb[:, k * h_dim + c2 * 128 : k * h_dim + (c2 + 1) * 128],
                        h1[:, k * b_total + bs : k * b_total + bs + bh],
                        start=(k == 0),
                        stop=(k == kh - 1),
                    )
            for c2 in range(kh):
                nc.scalar.activation(
                    h2[:, c2 * b_total + bs : c2 * b_total + bs + bh],
                    ps2[c2][:],
                    RELU,
                    bias=b2_sb[:, c2 : c2 + 1],
                )

            # ---- heads + softmax, per 128-box tile ----
            for t in range(bh // 128):
                bt = bs + t * 128
                pl = psum_pool.tile([128, C_OUT], F32, tag="acc", name=f"pl_{half}_{t}")
                for k in range(kh):
                    # stationary = h2 tile [K=ch2-slice, M=box], moving = Wcr
                    nc.tensor.matmul(
                        pl[:],
                        h2[:, k * b_total + bt : k * b_total + bt + 128],
                        wcr_sb[:, k * C_OUT : (k + 1) * C_OUT],
                        start=(k == 0),
                        stop=(k == kh - 1),
                    )
                logits = outp.tile([128, C_OUT], F32, tag="logits", name="logits")
                nc.vector.tensor_add(logits[:], pl[:], bcr_sb[:])
                # softmax over the 4 class columns
                negm = outp.tile([128, 1], F32, tag="negm", name="negm")
                nc.vector.tensor_reduce(negm[:], logits[:, 0:4], axis=AX, op=MAX, negate=True)
                e = outp.tile([128, 4], F32, tag="e", name="e")
                nc.scalar.activation(e[:], logits[:, 0:4], EXP, bias=negm[:])
                s = outp.tile([128, 1], F32, tag="s", name="s")
                nc.vector.tensor_reduce(s[:], e[:], axis=AX, op=ADD)
                r = outp.tile([128, 1], F32, tag="r", name="r")
                nc.vector.reciprocal(r[:], s[:])
                nc.vector.tensor_scalar_mul(logits[:, 0:4], e[:], r[:])
                nc.sync.dma_start(out[bt : bt + 128, :], logits[:])


_program_cache: dict = {}


def _get_program(key=(D_IN, H, B)) -> bass.Bass:
    if key not in _program_cache:
        _program_cache[key] = build_program(*key)
    return _program_cache[key]


def round_fp32r(a: np.ndarray) -> np.ndarray:
    """Round fp32 to the PE's fp32r format: e8m11, RNE at mantissa bit 12."""
    a = np.ascontiguousarray(a, dtype=np.float32)
    b = a.view(np.uint32)
    lsb = (b >> np.uint32(12)) & np.uint32(1)
    r = (b + np.uint32(0x7FF) + lsb) & np.uint32(0xFFFFF000)
    return r.view(np.float32)


def make_in_maps(inputs: dict) -> list[dict]:
    """Pad + shard the full inputs into per-core input maps."""
    x = np.asarray(inputs["feature_vectors"], dtype=np.float32)
    w1 = round_fp32r(np.asarray(inputs["W1"], dtype=np.float32))
    w2 = round_fp32r(np.asarray(inputs["W2"], dtype=np.float32))
    wc = np.asarray(inputs["Wc"], dtype=np.float32)
    wr = np.asarray(inputs["Wr"], dtype=np.float32)
    wcr = np.ascontiguousarray(np.concatenate([wc, wr], axis=1))  # [H, 16]
    b1 = np.ascontiguousarray(np.asarray(inputs["b1"], dtype=np.float32))
    b2 = np.ascontiguousarray(np.asarray(inputs["b2"], dtype=np.float32))
    bcr = np.concatenate(
        [np.asarray(inputs["bc"], np.float32), np.asarray(inputs["br"], np.float32)]
    )
    bcr = np.ascontiguousarray(np.tile(bcr[None, :], (128, 1)))  # [128, 16]

    n = x.shape[0]
    n_pad = N_CORES * B
    if n < n_pad:
        x = np.concatenate([x, np.zeros((n_pad - n, x.shape[1]), np.float32)], axis=0)

    in_maps = []
    for i in range(N_CORES):
        shard = x[i * B : (i + 1) * B]  # [B, D_IN]
        in_maps.append(
            {
                "xt": round_fp32r(shard.T),  # [D_IN, B]
                "w1": w1,
                "w2": w2,
                "wcr": wcr,
                "b1": b1,
                "b2": b2,
                "bcr": bcr,
            }
        )
    return in_maps


def run_spmd(inputs: dict, trace: bool = False):
    """Run on the 8 NeuronCores; returns (gathered [8192,16] array, results obj)."""
    nc = _get_program()
    in_maps = make_in_maps(inputs)
    res = run_bass_kernel_spmd(nc, in_maps, core_ids=list(range(N_CORES)), trace=trace)
    full = np.concatenate([res.results[i]["out"] for i in range(N_CORES)], axis=0)
    return full, res


def kernel(**inputs):
    full, _ = run_spmd(inputs, trace=False)
    n = np.asarray(inputs["feature_vectors"]).shape[0]
    out = full[:n]
    return out[:, :4].copy(), out[:, 4:].copy()
